# revision 27
# baseline (speedup 1.0000x reference)
"""ConvNeXt block (dwconv7 -> LN -> pwconv1 -> GELU -> GRN -> pwconv2 -> residual)
for Trainium2, batch-parallel across 8 NeuronCores (2 batches per core).

Self-contained: hardcodes shapes B=16, C=512, T=2048, I=1536, K=7.

v3: fully SBUF-resident, software-pipelined across batches.
  - x rows stay in SBUF per batch (conv taps slice them directly; residual
    reuses them); h ([I, T] f16, 48 KiB/partition) never leaves SBUF.
  - Stage s interleaves phase-2 of batch s-1 with phase-1 of batch s at
    t-chunk granularity so every engine's in-order stream pipelines.
  - Engine placement tuned to the TRN2 cost model: conv MACs (stt, no fast
    mode) + LN row math on DVE; GELU + gx square-accum + psum drains on ACT;
    y-normalize + w2 scaling + residual adds on Pool; matmuls dense on PE.
  - bf16 for y/ysq/w1pT (mm1), f16 for h/w2 (mm2): 1 cycle/row matmuls.

Math folding (host-side, weight-sized only):
  LN:  y_ln = (y - mu_t) * A_t * ln_g + ln_b      (A_t = rsqrt(var_t + eps))
  mm1: h_pre = w1p @ (A*y + B) + b1p, w1p = w1 * ln_g, b1p = b1 + w1 @ ln_b,
       B_t = -A_t * mu_t broadcast to all channel partitions
  GRN: h' = h * ss[i] + grn_b[i],  ss = 1 + grn_g * gx * d,
       gx = sqrt(sum_t h^2), d = 1/(mean_i gx + eps)
  mm2: out = (w2 * ss).T-contracted with h + (b2 + w2 @ grn_b) + residual
"""
import sys

sys.path.insert(0, "/opt/trn_rl_repo")

import numpy as np
import ml_dtypes
import concourse.bacc as bacc
import concourse.tile as tile
from concourse import mybir
from concourse.bass_utils import run_bass_kernel_spmd

F32 = mybir.dt.float32
F32R = mybir.dt.float32r
F16 = mybir.dt.float16
BF16 = mybir.dt.bfloat16
AF = mybir.ActivationFunctionType
OP = mybir.AluOpType

B, C, T, I, K = 16, 512, 2048, 1536, 7
NCORES = 8
BPC = B // NCORES          # batches per core
CC = C // 128              # 4 c-chunks
IC = I // 128              # 12 i-chunks
TC = T // 512              # 4 t-chunks
TN = 512                   # matmul free-dim tile
LN_EPS = 1e-6
GRN_EPS = 1e-6

_CACHE = {}


def _build(trace_sim=False, reps=1):
    nc = bacc.Bacc("TRN2", target_bir_lowering=False, debug=False,
                   num_devices=NCORES)
    dram = {}

    def din(name, shape, dt=F32):
        dram[name] = nc.dram_tensor(name, shape, dt, kind="ExternalInput").ap()
        return dram[name]

    x_d = din("xbf", [BPC, C, T], BF16)              # per-core batches (bf16)
    w1pT_d = din("w1pT", [C, I], BF16)               # (w1*ln_g).T  [c, i]
    b1p_d = din("b1p", [128, IC])                    # col-chunked b1p
    w2T_d = din("w2T", [I, C], F16)                  # w2.T  [i, c]
    b2p_d = din("b2p", [128, CC])                    # col-chunked b2 + w2@grn_b
    grng_d = din("grng", [128, IC])                  # col-chunked grn_g
    dww_d = din("dww", [128, CC, K])                 # depthwise taps per c-chunk
    dwb_d = din("dwb", [128, CC])                    # depthwise bias per c-chunk
    out_d = nc.dram_tensor("out", [BPC, C, T], BF16,
                           kind="ExternalOutput").ap()

    with tile.TileContext(nc, trace_sim=trace_sim) as tc:
        for _ in range(reps):
            _kernel_body(nc, tc, x_d, w1pT_d, b1p_d, w2T_d, b2p_d,
                         grng_d, dww_d, dwb_d, out_d)
    nc.compile()
    return nc


def _kernel_body(nc, tc, x_d, w1pT_d, b1p_d, w2T_d, b2p_d,
                 grng_d, dww_d, dwb_d, out_d):
    from contextlib import ExitStack
    ctx = ExitStack()
    with ctx:
        ctx.enter_context(nc.allow_low_precision(
            reason="bf16/f16 matmul operand rounding is intentional"))
        singles = ctx.enter_context(tc.tile_pool(name="singles", bufs=1))
        xp = ctx.enter_context(tc.tile_pool(name="xp", bufs=8))
        tpp = ctx.enter_context(tc.tile_pool(name="tpp", bufs=2))
        yp = ctx.enter_context(tc.tile_pool(name="yp", bufs=3))
        ysqp = ctx.enter_context(tc.tile_pool(name="ysqp", bufs=2))
        rowp = ctx.enter_context(tc.tile_pool(name="rowp", bufs=4))
        bcp = ctx.enter_context(tc.tile_pool(name="bcp", bufs=4))
        hp = ctx.enter_context(tc.tile_pool(name="hp", bufs=4))
        sqp = ctx.enter_context(tc.tile_pool(name="sqp", bufs=2))
        gxp = ctx.enter_context(tc.tile_pool(name="gxp", bufs=2))
        w2sp = ctx.enter_context(tc.tile_pool(name="w2sp", bufs=2))
        op_ = ctx.enter_context(tc.tile_pool(name="op", bufs=3))
        mm1ps = ctx.enter_context(tc.tile_pool(name="mm1ps", bufs=3,
                                               space="PSUM"))
        mm2ps = ctx.enter_context(tc.tile_pool(name="mm2ps", bufs=3,
                                               space="PSUM"))
        sps = ctx.enter_context(tc.tile_pool(name="sps", bufs=2, space="PSUM"))

        # ---- constants (conv taps first so conv starts immediately; the
        # x chunks for batch 0 are interleaved just after via load_x) ----
        dww = singles.tile([128, CC, K], F32)
        nc.sync.dma_start(dww[:], dww_d)
        dwb = singles.tile([128, CC], F32)
        nc.sync.dma_start(dwb[:], dwb_d)
        b1p = singles.tile([128, IC], F32)
        nc.sync.dma_start(b1p[:], b1p_d)
        w1pT = singles.tile([128, CC, I], BF16)
        w2m = singles.tile([128, IC, C], F16)
        b2p = singles.tile([128, CC], F32)
        grng = singles.tile([128, IC], F32)

        def load_weights():
            nc.sync.dma_start(w1pT[:],
                              w1pT_d.rearrange("(cc p) i -> p cc i", p=128))
            nc.sync.dma_start(w2m[:],
                              w2T_d.rearrange("(ic p) c -> p ic c", p=128))
            nc.sync.dma_start(b2p[:], b2p_d)
            nc.sync.dma_start(grng[:], grng_d)

        onesf = singles.tile([128, 1], F32)
        nc.vector.memset(onesf[:], 1.0)
        ones_col = singles.tile([128, 1], BF16)   # stats lhsT (K=128, M=1)
        nc.vector.tensor_copy(ones_col[:], onesf[:])
        onesrf = singles.tile([1, 128], F32)
        nc.vector.memset(onesrf[:], 1.0)
        ones_row = singles.tile([1, 128], F32R)   # bcast lhsT (K=1, M=128)
        nc.vector.tensor_copy(ones_row[:], onesrf[:])
        eps_ln = singles.tile([1, 1], F32)
        nc.vector.memset(eps_ln[:], LN_EPS)

        xv = x_d.rearrange("b (cc p) t -> b p cc t", p=128)

        # per-batch state carried across stages
        xres = {}       # b -> list of resident x row tiles
        ys = {}         # b -> list of per-t-chunk y tiles
        hts = {}        # b -> list of per-t-chunk h tiles
        gxparts = {}    # b -> gx^2 partial tile
        w2ss = {}       # b -> scaled w2

        def load_x(b):
            xres[b] = []
            for ci in range(CC):
                xr = xp.tile([128, T], BF16, tag="x", name=f"xr{b}_{ci}")
                eng = nc.sync if ci % 2 == 0 else nc.scalar
                if b == 0:
                    # small head segment first so conv(t0) starts ASAP
                    eng.dma_start(xr[:, 0:TN + 8], xv[b, :, ci, 0:TN + 8])
                else:
                    eng.dma_start(xr[:], xv[b, :, ci, :])
                xres[b].append(xr)
            if b == 0:
                for ci in range(CC):
                    eng = nc.sync if ci % 2 == 0 else nc.scalar
                    eng.dma_start(xres[b][ci][:, TN + 8:T],
                                  xv[b, :, ci, TN + 8:T])

        def conv_t(b, t):
            """depthwise conv for one t-chunk: per-ci bf16 tap products on DVE
            (4x mode) into two rolling planes, ci-merged adds on Pool.
            The very first chunk uses a pure-DVE MAC chain instead: lower
            latency (no DVE<->Pool ping-pong) to get PE started sooner."""
            t0 = t * TN
            if b == 0 and t == 0:
                y_t = yp.tile([128, CC, TN], BF16, tag="y", name=f"y{b}_{t}")
                ys[b].append(y_t)
                for ci in range(CC):
                    acc = y_t[:, ci, :]
                    xr = xres[b][ci]
                    nc.vector.tensor_scalar(acc, xr[:, t0:t0 + TN],
                                            dww[:, ci, 3:4],
                                            dwb[:, ci:ci + 1],
                                            OP.mult, OP.add)
                    for k in (2, 4, 1, 5, 0, 6):
                        d = k - 3
                        lo = max(0, -(t0 + d))
                        hi = min(TN, T - (t0 + d))
                        nc.vector.scalar_tensor_tensor(
                            acc[:, lo:hi], xr[:, t0 + d + lo:t0 + d + hi],
                            dww[:, ci, k:k + 1], acc[:, lo:hi],
                            OP.mult, OP.add)
                return
            y_t = yp.tile([128, CC, TN], BF16, tag="y", name=f"y{b}_{t}")
            ys[b].append(y_t)
            tp_ = tpp.tile([128, 2, CC, TN], BF16, tag="tp")

            def prod(plane, k):
                d = k - 3
                lo = max(0, -(t0 + d))
                hi = min(TN, T - (t0 + d))
                if lo > 0:
                    nc.vector.memset(tp_[:, plane, :, 0:lo], 0.0)
                if hi < TN:
                    nc.vector.memset(tp_[:, plane, :, hi:TN], 0.0)
                for ci in range(CC):
                    xr = xres[b][ci]
                    if k == 3:
                        nc.vector.tensor_scalar(
                            tp_[:, plane, ci, lo:hi],
                            xr[:, t0 + d + lo:t0 + d + hi],
                            dww[:, ci, k:k + 1], dwb[:, ci:ci + 1],
                            OP.mult, OP.add)
                    else:
                        nc.vector.tensor_scalar(
                            tp_[:, plane, ci, lo:hi],
                            xr[:, t0 + d + lo:t0 + d + hi],
                            dww[:, ci, k:k + 1], None, OP.mult)

            # in stage 0 (batch 0) Pool is idle: give it the off-chain adds
            aeng = nc.gpsimd if b == 0 else nc.vector
            a, bb = tp_[:, 0], tp_[:, 1]
            prod(0, 0)
            prod(1, 1)
            nc.vector.tensor_add(y_t[:], a, bb)
            prod(0, 2)
            prod(1, 3)
            aeng.tensor_add(a, a, bb)
            nc.vector.tensor_add(y_t[:], y_t[:], a)
            prod(0, 4)
            prod(1, 5)
            aeng.tensor_add(a, a, bb)
            nc.vector.tensor_add(y_t[:], y_t[:], a)
            prod(0, 6)
            nc.vector.tensor_add(y_t[:], y_t[:], a)

        def ln_t(b, t):
            """LN stats + row math + broadcast + in-place normalize of y."""
            y_t = ys[b][t]
            ysq = ysqp.tile([128, CC, TN], BF16, tag="ysq")
            nc.gpsimd.tensor_mul(ysq[:], y_t[:], y_t[:])
            sumy = sps.tile([1, TN], F32, tag="sp", name="sumy")
            sumsq = sps.tile([1, TN], F32, tag="sp", name="sumsq")
            for ci in range(CC):
                nc.tensor.matmul(sumy[:], ones_col[:], y_t[:, ci, :],
                                 start=(ci == 0), stop=(ci == CC - 1))
                nc.tensor.matmul(sumsq[:], ones_col[:], ysq[:, ci, :],
                                 start=(ci == 0), stop=(ci == CC - 1))
            mu = rowp.tile([1, TN], F32, tag="row")
            nc.scalar.activation(mu[:], sumy[:], AF.Copy, scale=1.0 / C)
            msq = rowp.tile([1, TN], F32, tag="row")
            nc.scalar.activation(msq[:], mu[:], AF.Square)
            var = rowp.tile([1, TN], F32, tag="row")
            nc.vector.scalar_tensor_tensor(var[:], sumsq[:], 1.0 / C,
                                           msq[:], OP.mult, OP.subtract)
            stdv = rowp.tile([1, TN], F32, tag="row")
            nc.scalar.activation(stdv[:], var[:], AF.Sqrt, bias=eps_ln[:])
            A_row = rowp.tile([1, TN], F32R, tag="row")
            nc.vector.reciprocal(A_row[:], stdv[:])
            B_row = rowp.tile([1, TN], F32R, tag="row")
            nc.vector.scalar_tensor_tensor(B_row[:], mu[:], -1.0,
                                           A_row[:].bitcast(F32),
                                           OP.mult, OP.mult)
            # broadcast A, B across partitions (K=1 matmuls), drain as bf16
            abc_ps = sps.tile([128, TN], F32, tag="sp", name="abc_ps")
            nc.tensor.matmul(abc_ps[:], ones_row[:], A_row[:],
                             start=True, stop=True)
            abc = bcp.tile([128, TN], BF16, tag="bc")
            nc.scalar.activation(abc[:], abc_ps[:], AF.Copy)
            bbc_ps = sps.tile([128, TN], F32, tag="sp", name="bbc_ps")
            nc.tensor.matmul(bbc_ps[:], ones_row[:], B_row[:],
                             start=True, stop=True)
            bbc = bcp.tile([128, TN], BF16, tag="bc")
            nc.scalar.activation(bbc[:], bbc_ps[:], AF.Copy)
            # normalize y in place on Pool: y = y*A + B
            for ci in range(CC):
                nc.gpsimd.tensor_mul(y_t[:, ci, :], y_t[:, ci, :], abc[:])
                nc.gpsimd.tensor_add(y_t[:, ci, :], y_t[:, ci, :], bbc[:])

        def mm1_t(b, t):
            """mm1 + GELU -> h (SBUF-resident), square-accum gx partials."""
            y_t = ys[b][t]
            h_t = hp.tile([128, IC, TN], F16, tag="h", name=f"h{b}_{t}")
            hts[b].append(h_t)
            gxpart = gxparts[b]
            for ii in range(IC):
                ph = mm1ps.tile([128, TN], F32, tag="mm1")
                isl = slice(ii * 128, (ii + 1) * 128)
                for ci in range(CC):
                    nc.tensor.matmul(ph[:], w1pT[:, ci, isl],
                                     y_t[:, ci, :],
                                     start=(ci == 0), stop=(ci == CC - 1))
                nc.scalar.activation(h_t[:, ii, :], ph[:], AF.Gelu,
                                     bias=b1p[:, ii:ii + 1])
                sq = sqp.tile([128, TN], F16, tag="sq")
                use_act = (ii % 2 == 0) or (b == 0 and ii == 11)
                if use_act:       # ACT square + free-dim accumulate
                    nc.scalar.activation(sq[:], h_t[:, ii, :], AF.Square,
                                         accum_out=gxpart[:, ii, t:t + 1])
                else:             # DVE square + accumulate
                    nc.vector.scalar_tensor_tensor(
                        sq[:], h_t[:, ii, :], 1.0, h_t[:, ii, :],
                        OP.bypass, OP.mult,
                        accum_out=gxpart[:, ii, t:t + 1])

        def grn(b):
            """GRN scale factors + scaled w2 copy for batch b."""
            gxpart = gxparts[b]
            gxsq = gxp.tile([128, IC], F32, tag="gx2")
            nc.vector.tensor_reduce(gxsq[:], gxpart[:],
                                    axis=mybir.AxisListType.X, op=OP.add)
            gx = gxp.tile([128, IC], BF16, tag="gx2")
            nc.scalar.activation(gx[:], gxsq[:], AF.Sqrt)
            gsum = sps.tile([1, IC], F32, tag="sp", name="gsum")
            nc.tensor.matmul(gsum[:], ones_col[:], gx[:], start=True,
                             stop=True)
            gtot = gxp.tile([1, 1], F32, tag="gx3")
            nc.vector.tensor_reduce(gtot[:], gsum[:],
                                    axis=mybir.AxisListType.X, op=OP.add)
            dinv = gxp.tile([1, 1], F32, tag="gx3")
            nc.vector.tensor_scalar(dinv[:], gtot[:], 1.0 / I, GRN_EPS,
                                    OP.mult, OP.add)
            d_row = gxp.tile([1, 1], F32, tag="gx3")
            nc.vector.reciprocal(d_row[:], dinv[:])
            dbc = gxp.tile([128, 1], F32, tag="gx4")
            nc.gpsimd.partition_broadcast(dbc[:], d_row[:])
            ss = gxp.tile([128, IC], F32, tag="gx4")
            nc.vector.scalar_tensor_tensor(ss[:], gx[:], dbc[:],
                                           grng[:], OP.mult, OP.mult)
            nc.vector.tensor_scalar(ss[:], ss[:], 1.0, None, OP.add)
            w2s = w2sp.tile([128, IC, C], F16, tag="w2s", name=f"w2s{b}")
            w2ss[b] = w2s
            for ii in range(IC):
                nc.gpsimd.tensor_scalar(w2s[:, ii, :], w2m[:, ii, :],
                                        ss[:, ii:ii + 1], None, OP.mult)

        def phase2_t(b, t):
            """mm2 + bias + residual + store for one t-chunk of batch b."""
            ts_ = slice(t * TN, (t + 1) * TN)
            h_t = hts[b][t]
            w2s = w2ss[b]
            for ci in range(CC):
                po = mm2ps.tile([128, TN], F32, tag="mm2")
                csl = slice(ci * 128, (ci + 1) * 128)
                for ii in range(IC):
                    nc.tensor.matmul(po[:], w2s[:, ii, csl],
                                     h_t[:, ii, :],
                                     start=(ii == 0), stop=(ii == IC - 1))
                o_sb = op_.tile([128, TN], BF16, tag="o")
                nc.scalar.activation(o_sb[:], po[:], AF.Identity,
                                     bias=b2p[:, ci:ci + 1])
                nc.gpsimd.tensor_add(o_sb[:], o_sb[:], xres[b][ci][:, ts_])
                nc.sync.dma_start(
                    out_d[b, ci * 128:(ci + 1) * 128, ts_], o_sb[:])

        # ---- software-pipelined schedule ----
        # Stage s interleaves phase-2 of batch s-1 with phase-1 of batch s;
        # within phase 1, mm1(t) lags conv/LN by one t-chunk so the next
        # chunk's LN row chain is queued ahead of this chunk's GELU burst.
        # Exception: stage 0 emits mm1(t0) immediately (no ACT backlog yet)
        # so PE has work while the second chunk's conv is still in flight.
        load_x(0)
        load_weights()
        for s in range(BPC + 1):
            if s < BPC:
                hts[s] = []
                ys[s] = []
                gxparts[s] = gxp.tile([128, IC, TC], F32, tag="gxpart",
                                      name=f"gxpart{s}")
            for t in range(TC):
                if s > 0:
                    phase2_t(s - 1, t)
                if s < BPC:
                    conv_t(s, t)
                    ln_t(s, t)
                    if s == 0 and t == 0:
                        mm1_t(s, 0)
                    elif s == 0 and t == 1:
                        pass
                    elif t > 0:
                        mm1_t(s, t - 1)
            if s < BPC:
                mm1_t(s, TC - 1)
                grn(s)
            if s + 1 < BPC:
                load_x(s + 1)


def _host_prep(inputs):
    w1 = inputs["w1"].astype(np.float64)
    ln_g = inputs["ln_g"].astype(np.float64)
    ln_b = inputs["ln_b"].astype(np.float64)
    w2 = inputs["w2"].astype(np.float64)
    w1p = w1 * ln_g[None, :]                         # [I, C]
    prep = {
        "w1pT": np.ascontiguousarray(w1p.T).astype(ml_dtypes.bfloat16),
        "b1p": (inputs["b1"].astype(np.float64) + w1 @ ln_b)
               .astype(np.float32).reshape(IC, 128).T.copy(),
        "w2T": np.ascontiguousarray(w2.T).astype(np.float16),
        "b2p": (inputs["b2"].astype(np.float64)
                + w2 @ inputs["grn_b"].astype(np.float64))
               .astype(np.float32).reshape(CC, 128).T.copy(),
        "grng": inputs["grn_g"].reshape(IC, 128).T.copy().astype(np.float32),
        "dww": inputs["dw_w"].reshape(C, K).reshape(CC, 128, K)
               .transpose(1, 0, 2).copy().astype(np.float32),
        "dwb": inputs["dw_b"].reshape(CC, 128).T.copy().astype(np.float32),
    }
    return prep


def _percore_maps(inputs):
    prep = _host_prep(inputs)
    xbf = np.asarray(inputs["x"]).astype(ml_dtypes.bfloat16)
    in_maps = []
    for c in range(NCORES):
        m = dict(prep)
        m["xbf"] = np.ascontiguousarray(xbf[c * BPC:(c + 1) * BPC])
        in_maps.append(m)
    return in_maps


def run(inputs, trace=False, **kw):
    if "nc" not in _CACHE:
        _CACHE["nc"] = _build()
    nc = _CACHE["nc"]
    in_maps = _percore_maps(inputs)
    res = run_bass_kernel_spmd(nc, in_maps, core_ids=list(range(NCORES)),
                               trace=trace, **kw)
    out = np.concatenate(
        [np.asarray(r["out"]).astype(np.float32) for r in res.results], axis=0)
    return out, res


def kernel(**inputs):
    out, _ = run(inputs)
    return out
